# revision 4
# baseline (speedup 1.0000x reference)
"""BinomialLoss pair loss/grad kernel for 8 trn2 NeuronCores.

Strategy: rows AND columns of sim_mat are permuted (host-side) into
class-sorted order (perm = argsort(targets)).  Row-wise sharding across 8
cores.  In this layout the "same-class" pairs of each 128-row block live in
one narrow contiguous column band (the diag slab), so the dense per-element
pass is branch-free:

  loss = softplus(40x-20) = 40*max(x,0.5)-20 + ln(1+exp(-40|x-0.5|))
  grad = gn * sigmoid(40x-20) = gnh*tanh(20x-10)+gnh     (gnh = 20*rv/neg_cnt)

The slab pass recomputes the pos-branch values (softplus(-2x+1), sigmoid) for
the band, along with pos_cnt = rowsum((x<1)*same_class); the host scatters
those values over the same-class positions of the dense output.
"""
import sys
sys.path.insert(0, "/opt/trn_rl_repo")
import numpy as np

N = 8192
NCORES = 8
RPC = N // NCORES          # rows per core = 1024
NBLK = RPC // 128          # 8 blocks of 128 rows per core
CHUNK = 2048
NCHUNK = N // CHUNK        # 4 chunks per block
XSPAN = 4096               # input DMA granularity (2 chunks)
ALPHA, BETA, MARGIN = 40.0, 2.0, 0.5

_prog_cache = {}


def _build_program(WD):
    import concourse.bacc as bacc
    import concourse.mybir as mybir
    import concourse.tile as tile

    F32 = mybir.dt.float32
    AF = mybir.ActivationFunctionType
    OP = mybir.AluOpType

    nc = bacc.Bacc("TRN2", target_bir_lowering=False, debug=False,
                   num_devices=NCORES)
    x_d = nc.dram_tensor("x", [RPC, N], F32, kind="ExternalInput")
    xd_d = nc.dram_tensor("xd", [RPC, WD], F32, kind="ExternalInput")
    eqd_d = nc.dram_tensor("eqd", [RPC, WD], F32, kind="ExternalInput")
    gnh_d = nc.dram_tensor("gnh", [128, NBLK], F32, kind="ExternalInput")
    rv_d = nc.dram_tensor("rv", [128, NBLK], F32, kind="ExternalInput")
    m2rv_d = nc.dram_tensor("m2rv", [128, NBLK], F32, kind="ExternalInput")
    loss_d = nc.dram_tensor("loss", [RPC, N], F32, kind="ExternalOutput")
    grad_d = nc.dram_tensor("grad", [RPC, N], F32, kind="ExternalOutput")
    lossd_d = nc.dram_tensor("lossd", [RPC, WD], F32, kind="ExternalOutput")
    gradd_d = nc.dram_tensor("gradd", [RPC, WD], F32, kind="ExternalOutput")

    with tile.TileContext(nc) as tc:
        with tc.tile_pool(name="const", bufs=1) as cp, \
             tc.tile_pool(name="main", bufs=2) as mp, \
             tc.tile_pool(name="one", bufs=1) as op1, \
             tc.tile_pool(name="slab", bufs=2) as sp, \
             tc.tile_pool(name="tiny", bufs=2) as tp:
            bm05 = cp.tile([128, 1], F32)
            nc.vector.memset(bm05[:], -0.5)
            bm10 = cp.tile([128, 1], F32)
            nc.vector.memset(bm10[:], -10.0)
            bp05 = cp.tile([128, 1], F32)
            nc.vector.memset(bp05[:], 0.5)
            gnh_t = cp.tile([128, NBLK], F32)
            nc.sync.dma_start(out=gnh_t[:], in_=gnh_d[:])
            rv_t = cp.tile([128, NBLK], F32)
            nc.sync.dma_start(out=rv_t[:], in_=rv_d[:])
            m2rv_t = cp.tile([128, NBLK], F32)
            nc.sync.dma_start(out=m2rv_t[:], in_=m2rv_d[:])

            for b in range(NBLK):
                r0 = b * 128
                gnh_ap = gnh_t[:, b:b + 1]
                rv_ap = rv_t[:, b:b + 1]
                m2rv_ap = m2rv_t[:, b:b + 1]

                # ---------- dense pass over the full row block ----------
                for xi in range(N // XSPAN):
                    xs0 = xi * XSPAN
                    xin = mp.tile([128, XSPAN], F32, tag="x")
                    nc.sync.dma_start(out=xin[:], in_=x_d[r0:r0 + 128, xs0:xs0 + XSPAN])
                    for cj in range(XSPAN // CHUNK):
                        c0 = xs0 + cj * CHUNK
                        x = xin[:, cj * CHUNK:(cj + 1) * CHUNK]
                        d = mp.tile([128, CHUNK], F32, tag="d")
                        nc.gpsimd.tensor_scalar(d[:], x, 0.5, None, OP.subtract)
                        t0 = mp.tile([128, CHUNK], F32, tag="t0")
                        nc.vector.scalar_tensor_tensor(t0[:], d[:], -1.0, d[:],
                                                       OP.mult, OP.min)  # -|x-0.5|
                        ep = op1.tile([128, CHUNK], F32, tag="ep")
                        nc.scalar.activation(ep[:], t0[:], AF.Exp, bias=0.0, scale=40.0)
                        l1p = op1.tile([128, CHUNK], F32, tag="l1p")
                        nc.scalar.activation(l1p[:], ep[:], AF.Ln, bias=1.0, scale=1.0)
                        spt = op1.tile([128, CHUNK], F32, tag="spt")
                        nc.vector.tensor_scalar(spt[:], x, 0.5, ALPHA, OP.max, OP.mult)
                        loss = mp.tile([128, CHUNK], F32, tag="loss")
                        nc.vector.scalar_tensor_tensor(loss[:], spt[:], -20.0, l1p[:],
                                                       OP.add, OP.add)
                        nc.sync.dma_start(out=loss_d[r0:r0 + 128, c0:c0 + CHUNK],
                                          in_=loss[:])
                        th = mp.tile([128, CHUNK], F32, tag="th")
                        nc.scalar.activation(th[:], x, AF.Tanh, bias=bm10[:], scale=20.0)
                        grad = mp.tile([128, CHUNK], F32, tag="grad")
                        nc.gpsimd.tensor_scalar(grad[:], th[:], gnh_ap, gnh_ap,
                                                OP.mult, OP.add)
                        nc.sync.dma_start(out=grad_d[r0:r0 + 128, c0:c0 + CHUNK],
                                          in_=grad[:])

                # ---------- diag slab pass ----------
                xd = sp.tile([128, WD], F32, tag="xd")
                nc.sync.dma_start(out=xd[:], in_=xd_d[r0:r0 + 128, :])
                eqd = sp.tile([128, WD], F32, tag="eqd")
                nc.sync.dma_start(out=eqd[:], in_=eqd_d[r0:r0 + 128, :])

                td = sp.tile([128, WD], F32, tag="td")
                nc.vector.tensor_scalar(td[:], xd[:], 1.0, rv_ap, OP.is_lt, OP.mult)
                tde = sp.tile([128, WD], F32, tag="tde")
                pc = tp.tile([128, 1], F32, tag="pc")
                nc.vector.tensor_mul(tde[:], td[:], eqd[:])
                nc.vector.tensor_reduce(pc[:], tde[:], mybir.AxisListType.X, OP.add)
                t0d = sp.tile([128, WD], F32, tag="t0d")
                nc.scalar.activation(t0d[:], xd[:], AF.Abs, bias=bm05[:], scale=1.0)
                epd = sp.tile([128, WD], F32, tag="epd")
                nc.scalar.activation(epd[:], t0d[:], AF.Exp, bias=0.0, scale=-2.0)
                l1pd = sp.tile([128, WD], F32, tag="l1pd")
                nc.scalar.activation(l1pd[:], epd[:], AF.Ln, bias=1.0, scale=1.0)
                sptd = sp.tile([128, WD], F32, tag="sptd")
                nc.vector.tensor_scalar(sptd[:], xd[:], 0.5, -BETA, OP.min, OP.mult)
                lossd_pre = sp.tile([128, WD], F32, tag="lossd_pre")
                nc.vector.scalar_tensor_tensor(lossd_pre[:], sptd[:], 1.0, l1pd[:],
                                               OP.add, OP.add)  # softplus(-2x+1)
                lossd = sp.tile([128, WD], F32, tag="lossd")
                nc.vector.tensor_mul(lossd[:], lossd_pre[:], td[:])
                nc.sync.dma_start(out=lossd_d[r0:r0 + 128, :], in_=lossd[:])

                thd = sp.tile([128, WD], F32, tag="thd")
                nc.scalar.activation(thd[:], xd[:], AF.Tanh, bias=bp05[:], scale=-1.0)
                pc2 = tp.tile([128, 1], F32, tag="pc2")
                nc.vector.tensor_scalar(pc2[:], pc[:], 1.0, None, OP.max)
                rcp = tp.tile([128, 1], F32, tag="rcp")
                nc.vector.reciprocal(rcp[:], pc2[:])
                gph = tp.tile([128, 1], F32, tag="gph")
                nc.vector.tensor_scalar(gph[:], rcp[:], m2rv_ap, 0.5, OP.mult, OP.mult)
                gd1 = sp.tile([128, WD], F32, tag="gd1")
                nc.vector.tensor_scalar(gd1[:], thd[:], gph[:], gph[:], OP.mult, OP.add)
                gradd = sp.tile([128, WD], F32, tag="gradd")
                nc.vector.scalar_tensor_tensor(gradd[:], gd1[:], 1.0, td[:],
                                               OP.mult, OP.mult)
                nc.sync.dma_start(out=gradd_d[r0:r0 + 128, :], in_=gradd[:])

    nc.compile()
    return nc


def _prepare(sim_mat, targets):
    """Host-side geometry + per-core input maps."""
    t = np.asarray(targets)
    x = np.ascontiguousarray(np.asarray(sim_mat, dtype=np.float32))
    perm = np.argsort(t, kind="stable")
    ts = t[perm]                                   # sorted targets
    nclass = int(ts.max()) + 1 if len(ts) else 1
    cs = np.searchsorted(ts, np.arange(nclass))         # class start
    ce = np.searchsorted(ts, np.arange(nclass), side="right")  # class end
    hist = ce - cs

    neg_raw = N - hist[ts]                         # per sorted row
    rv = (neg_raw > 0).astype(np.float32)
    ncnt = np.maximum(neg_raw, 1).astype(np.float64)
    gnh = (20.0 * rv / ncnt).astype(np.float32)
    m2rv = (-2.0 * rv).astype(np.float32)

    # block geometry: slab col range per (core, block)
    W0 = np.empty(NCORES * NBLK, dtype=np.int64)
    W1 = np.empty(NCORES * NBLK, dtype=np.int64)
    for blk in range(NCORES * NBLK):
        r0 = blk * 128
        W0[blk] = cs[ts[r0]]
        W1[blk] = ce[ts[r0 + 127]]
    WD = int(((W1 - W0).max() + 15) // 16 * 16)

    sim_perm = x[perm][:, perm]                    # class-sorted both ways

    in_maps = []
    for k in range(NCORES):
        rs = slice(k * RPC, (k + 1) * RPC)
        xk = np.ascontiguousarray(sim_perm[rs])
        xd = np.full((RPC, WD), 2.0, dtype=np.float32)
        eqd = np.zeros((RPC, WD), dtype=np.float32)
        for b in range(NBLK):
            blk = k * NBLK + b
            w0, w1 = W0[blk], W1[blk]
            span = w1 - w0
            rows = slice(b * 128, (b + 1) * 128)
            xd[rows, :span] = xk[rows, w0:w1]
            tb = ts[k * RPC + b * 128:k * RPC + (b + 1) * 128]   # [128]
            eqd[rows, :span] = (tb[:, None] == ts[w0:w1][None, :]).astype(np.float32)

        def fold(vec):  # [RPC] -> [128, NBLK] with [p, b] = vec[b*128+p]
            return np.ascontiguousarray(
                vec[k * RPC:(k + 1) * RPC].reshape(NBLK, 128).T)

        in_maps.append({
            "x": xk, "xd": xd, "eqd": eqd,
            "gnh": fold(gnh), "rv": fold(rv), "m2rv": fold(m2rv),
        })
    return perm, ts, cs, ce, hist, rv, W0, W1, WD, in_maps


def _assemble(results, perm, ts, cs, ce, hist, rv, W0, W1, WD):
    loss_p = np.vstack([results[k]["loss"] for k in range(NCORES)])
    grad_p = np.vstack([results[k]["grad"] for k in range(NCORES)])
    lossd = np.vstack([results[k]["lossd"] for k in range(NCORES)])
    gradd = np.vstack([results[k]["gradd"] for k in range(NCORES)])

    # scatter same-class (diag band) values over the dense outputs
    L = hist[ts]                                   # band length per sorted row
    rows_rep = np.repeat(np.arange(N), L)
    band_off = np.concatenate([[0], np.cumsum(L)])[:-1]
    idx = np.arange(L.sum()) - np.repeat(band_off, L)       # 0..L[r)-1 within row
    jj = cs[ts[rows_rep]] + idx                   # sorted-space column
    kk = jj - W0[rows_rep // 128]                 # column within slab
    loss_p[rows_rep, jj] = lossd[rows_rep, kk]
    grad_p[rows_rep, jj] = gradd[rows_rep, kk]

    if not rv.all():                               # rows with no negatives: loss = 0
        loss_p[rv == 0.0, :] = 0.0

    out_loss = np.empty((N, N), dtype=np.float32)
    out_grad = np.empty((N, N), dtype=np.float32)
    pix = np.ix_(perm, perm)
    out_loss[pix] = loss_p
    out_grad[pix] = grad_p
    return out_loss.reshape(-1), out_grad.reshape(-1)


def run(sim_mat, targets, trace=False):
    from concourse.bass_utils import run_bass_kernel_spmd
    perm, ts, cs, ce, hist, rv, W0, W1, WD, in_maps = _prepare(sim_mat, targets)
    if WD not in _prog_cache:
        _prog_cache[WD] = _build_program(WD)
    nc = _prog_cache[WD]
    res = run_bass_kernel_spmd(nc, in_maps, list(range(NCORES)), trace=trace)
    outs = _assemble(res.results, perm, ts, cs, ce, hist, rv, W0, W1, WD)
    return outs, res.exec_time_ns


def kernel(sim_mat, targets):
    outs, _ = run(sim_mat, targets, trace=False)
    return outs


# revision 5
# speedup vs baseline: 3.1762x; 3.1762x over previous
"""BinomialLoss pair loss/grad kernel for 8 trn2 NeuronCores.

Strategy: rows AND columns of sim_mat are permuted (host-side) into
class-sorted order (perm = argsort(targets)).  Row-wise sharding across 8
cores.  In this layout the "same-class" pairs of each 128-row block live in
one narrow contiguous column band (the diag slab), so the dense per-element
pass is branch-free:

  loss = softplus(40x-20) = 40*max(x,0.5)-20 + ln(1+exp(-40|x-0.5|))
  grad = gn * sigmoid(40x-20) = gnh*tanh(20x-10)+gnh     (gnh = 20*rv/neg_cnt)

The slab pass recomputes the pos-branch values (softplus(-2x+1), sigmoid) for
the band, along with pos_cnt = rowsum((x<1)*same_class); the host scatters
those values over the same-class positions of the dense output.
"""
import sys
sys.path.insert(0, "/opt/trn_rl_repo")
import numpy as np

N = 8192
NCORES = 8
RPC = N // NCORES          # rows per core = 1024
NBLK = RPC // 128          # 8 blocks of 128 rows per core
CHUNK = 2048
NCHUNK = N // CHUNK        # 4 chunks per block
XSPAN = 4096               # input DMA granularity (2 chunks)
ALPHA, BETA, MARGIN = 40.0, 2.0, 0.5
ABS_DVE_CHUNKS = (1, 3)    # chunks whose |x-0.5| is computed on DVE (load balance)

_prog_cache = {}


def _build_program(WD):
    import concourse.bacc as bacc
    import concourse.mybir as mybir
    import concourse.tile as tile

    F32 = mybir.dt.float32
    AF = mybir.ActivationFunctionType
    OP = mybir.AluOpType

    nc = bacc.Bacc("TRN2", target_bir_lowering=False, debug=False,
                   num_devices=NCORES)
    x_d = nc.dram_tensor("x", [RPC, N], F32, kind="ExternalInput")
    xd_d = nc.dram_tensor("xd", [RPC, WD], F32, kind="ExternalInput")
    eqd_d = nc.dram_tensor("eqd", [RPC, WD], F32, kind="ExternalInput")
    gnh_d = nc.dram_tensor("gnh", [128, NBLK], F32, kind="ExternalInput")
    rv_d = nc.dram_tensor("rv", [128, NBLK], F32, kind="ExternalInput")
    m2rv_d = nc.dram_tensor("m2rv", [128, NBLK], F32, kind="ExternalInput")
    loss_d = nc.dram_tensor("loss", [RPC, N], F32, kind="ExternalOutput")
    grad_d = nc.dram_tensor("grad", [RPC, N], F32, kind="ExternalOutput")
    lossd_d = nc.dram_tensor("lossd", [RPC, WD], F32, kind="ExternalOutput")
    gradd_d = nc.dram_tensor("gradd", [RPC, WD], F32, kind="ExternalOutput")

    with tile.TileContext(nc) as tc:
        with tc.tile_pool(name="const", bufs=1) as cp, \
             tc.tile_pool(name="xin", bufs=3) as xp, \
             tc.tile_pool(name="main", bufs=2) as mp, \
             tc.tile_pool(name="one", bufs=1) as op1, \
             tc.tile_pool(name="slab", bufs=2) as sp, \
             tc.tile_pool(name="tiny", bufs=2) as tp:
            bm05 = cp.tile([128, 1], F32)
            nc.vector.memset(bm05[:], -0.5)
            bm10 = cp.tile([128, 1], F32)
            nc.vector.memset(bm10[:], -10.0)
            bp05 = cp.tile([128, 1], F32)
            nc.vector.memset(bp05[:], 0.5)
            gnh_t = cp.tile([128, NBLK], F32)
            nc.sync.dma_start(out=gnh_t[:], in_=gnh_d[:])
            rv_t = cp.tile([128, NBLK], F32)
            nc.sync.dma_start(out=rv_t[:], in_=rv_d[:])
            m2rv_t = cp.tile([128, NBLK], F32)
            nc.sync.dma_start(out=m2rv_t[:], in_=m2rv_d[:])

            for b in range(NBLK):
                r0 = b * 128
                gnh_ap = gnh_t[:, b:b + 1]
                rv_ap = rv_t[:, b:b + 1]
                m2rv_ap = m2rv_t[:, b:b + 1]

                # ---------- dense pass over the full row block ----------
                for xi in range(N // XSPAN):
                    xs0 = xi * XSPAN
                    xin = xp.tile([128, XSPAN], F32, tag="x")
                    nc.sync.dma_start(out=xin[:], in_=x_d[r0:r0 + 128, xs0:xs0 + XSPAN])
                    for cj in range(XSPAN // CHUNK):
                        ci = xi * (XSPAN // CHUNK) + cj
                        c0 = xs0 + cj * CHUNK
                        x = xin[:, cj * CHUNK:(cj + 1) * CHUNK]
                        ep = op1.tile([128, CHUNK], F32, tag="ep")
                        if ci in ABS_DVE_CHUNKS:
                            d = mp.tile([128, CHUNK], F32, tag="d")
                            nc.vector.tensor_scalar(d[:], x, 0.5, None, OP.subtract)
                            t0 = mp.tile([128, CHUNK], F32, tag="t0")
                            nc.vector.scalar_tensor_tensor(t0[:], d[:], -1.0, d[:],
                                                           OP.mult, OP.min)  # -|x-0.5|
                            nc.scalar.activation(ep[:], t0[:], AF.Exp, bias=0.0,
                                                 scale=40.0)
                        else:
                            t0 = mp.tile([128, CHUNK], F32, tag="t0")
                            nc.scalar.activation(t0[:], x, AF.Abs, bias=bm05[:],
                                                 scale=1.0)  # |x-0.5|
                            nc.scalar.activation(ep[:], t0[:], AF.Exp, bias=0.0,
                                                 scale=-40.0)
                        l1p = op1.tile([128, CHUNK], F32, tag="l1p")
                        nc.scalar.activation(l1p[:], ep[:], AF.Ln, bias=1.0, scale=1.0)
                        spt = op1.tile([128, CHUNK], F32, tag="spt")
                        nc.vector.tensor_scalar(spt[:], x, 0.5, ALPHA, OP.max, OP.mult)
                        loss = mp.tile([128, CHUNK], F32, tag="loss")
                        nc.vector.scalar_tensor_tensor(loss[:], spt[:], -20.0, l1p[:],
                                                       OP.add, OP.add)
                        nc.scalar.dma_start(out=loss_d[r0:r0 + 128, c0:c0 + CHUNK],
                                            in_=loss[:])
                        th = mp.tile([128, CHUNK], F32, tag="th")
                        nc.scalar.activation(th[:], x, AF.Tanh, bias=bm10[:], scale=20.0)
                        grad = mp.tile([128, CHUNK], F32, tag="grad")
                        nc.vector.tensor_scalar(grad[:], th[:], gnh_ap, gnh_ap,
                                                OP.mult, OP.add)
                        nc.scalar.dma_start(out=grad_d[r0:r0 + 128, c0:c0 + CHUNK],
                                            in_=grad[:])

                # ---------- diag slab pass ----------
                xd = sp.tile([128, WD], F32, tag="xd")
                nc.sync.dma_start(out=xd[:], in_=xd_d[r0:r0 + 128, :])
                eqd = sp.tile([128, WD], F32, tag="eqd")
                nc.sync.dma_start(out=eqd[:], in_=eqd_d[r0:r0 + 128, :])

                td = sp.tile([128, WD], F32, tag="td")
                nc.vector.tensor_scalar(td[:], xd[:], 1.0, rv_ap, OP.is_lt, OP.mult)
                tde = sp.tile([128, WD], F32, tag="tde")
                pc = tp.tile([128, 1], F32, tag="pc")
                nc.vector.tensor_mul(tde[:], td[:], eqd[:])
                nc.vector.tensor_reduce(pc[:], tde[:], mybir.AxisListType.X, OP.add)
                t0d = sp.tile([128, WD], F32, tag="t0d")
                nc.scalar.activation(t0d[:], xd[:], AF.Abs, bias=bm05[:], scale=1.0)
                epd = sp.tile([128, WD], F32, tag="epd")
                nc.scalar.activation(epd[:], t0d[:], AF.Exp, bias=0.0, scale=-2.0)
                l1pd = sp.tile([128, WD], F32, tag="l1pd")
                nc.scalar.activation(l1pd[:], epd[:], AF.Ln, bias=1.0, scale=1.0)
                sptd = sp.tile([128, WD], F32, tag="sptd")
                nc.vector.tensor_scalar(sptd[:], xd[:], 0.5, -BETA, OP.min, OP.mult)
                lossd_pre = sp.tile([128, WD], F32, tag="lossd_pre")
                nc.vector.scalar_tensor_tensor(lossd_pre[:], sptd[:], 1.0, l1pd[:],
                                               OP.add, OP.add)  # softplus(-2x+1)
                lossd = sp.tile([128, WD], F32, tag="lossd")
                nc.vector.tensor_mul(lossd[:], lossd_pre[:], td[:])
                nc.scalar.dma_start(out=lossd_d[r0:r0 + 128, :], in_=lossd[:])

                thd = sp.tile([128, WD], F32, tag="thd")
                nc.scalar.activation(thd[:], xd[:], AF.Tanh, bias=bp05[:], scale=-1.0)
                pc2 = tp.tile([128, 1], F32, tag="pc2")
                nc.vector.tensor_scalar(pc2[:], pc[:], 1.0, None, OP.max)
                rcp = tp.tile([128, 1], F32, tag="rcp")
                nc.vector.reciprocal(rcp[:], pc2[:])
                gph = tp.tile([128, 1], F32, tag="gph")
                nc.vector.tensor_scalar(gph[:], rcp[:], m2rv_ap, 0.5, OP.mult, OP.mult)
                gd1 = sp.tile([128, WD], F32, tag="gd1")
                nc.vector.tensor_scalar(gd1[:], thd[:], gph[:], gph[:], OP.mult, OP.add)
                gradd = sp.tile([128, WD], F32, tag="gradd")
                nc.vector.scalar_tensor_tensor(gradd[:], gd1[:], 1.0, td[:],
                                               OP.mult, OP.mult)
                nc.scalar.dma_start(out=gradd_d[r0:r0 + 128, :], in_=gradd[:])

    nc.compile()
    return nc


def _prepare(sim_mat, targets):
    """Host-side geometry + per-core input maps."""
    t = np.asarray(targets)
    x = np.ascontiguousarray(np.asarray(sim_mat, dtype=np.float32))
    perm = np.argsort(t, kind="stable")
    ts = t[perm]                                   # sorted targets
    nclass = int(ts.max()) + 1 if len(ts) else 1
    cs = np.searchsorted(ts, np.arange(nclass))         # class start
    ce = np.searchsorted(ts, np.arange(nclass), side="right")  # class end
    hist = ce - cs

    neg_raw = N - hist[ts]                         # per sorted row
    rv = (neg_raw > 0).astype(np.float32)
    ncnt = np.maximum(neg_raw, 1).astype(np.float64)
    gnh = (20.0 * rv / ncnt).astype(np.float32)
    m2rv = (-2.0 * rv).astype(np.float32)

    # block geometry: slab col range per (core, block)
    W0 = np.empty(NCORES * NBLK, dtype=np.int64)
    W1 = np.empty(NCORES * NBLK, dtype=np.int64)
    for blk in range(NCORES * NBLK):
        r0 = blk * 128
        W0[blk] = cs[ts[r0]]
        W1[blk] = ce[ts[r0 + 127]]
    WD = int(((W1 - W0).max() + 15) // 16 * 16)

    sim_perm = x[perm][:, perm]                    # class-sorted both ways

    in_maps = []
    for k in range(NCORES):
        rs = slice(k * RPC, (k + 1) * RPC)
        xk = np.ascontiguousarray(sim_perm[rs])
        xd = np.full((RPC, WD), 2.0, dtype=np.float32)
        eqd = np.zeros((RPC, WD), dtype=np.float32)
        for b in range(NBLK):
            blk = k * NBLK + b
            w0, w1 = W0[blk], W1[blk]
            span = w1 - w0
            rows = slice(b * 128, (b + 1) * 128)
            xd[rows, :span] = xk[rows, w0:w1]
            tb = ts[k * RPC + b * 128:k * RPC + (b + 1) * 128]   # [128]
            eqd[rows, :span] = (tb[:, None] == ts[w0:w1][None, :]).astype(np.float32)

        def fold(vec):  # [RPC] -> [128, NBLK] with [p, b] = vec[b*128+p]
            return np.ascontiguousarray(
                vec[k * RPC:(k + 1) * RPC].reshape(NBLK, 128).T)

        in_maps.append({
            "x": xk, "xd": xd, "eqd": eqd,
            "gnh": fold(gnh), "rv": fold(rv), "m2rv": fold(m2rv),
        })
    return perm, ts, cs, ce, hist, rv, W0, W1, WD, in_maps


def _assemble(results, perm, ts, cs, ce, hist, rv, W0, W1, WD):
    loss_p = np.vstack([results[k]["loss"] for k in range(NCORES)])
    grad_p = np.vstack([results[k]["grad"] for k in range(NCORES)])
    lossd = np.vstack([results[k]["lossd"] for k in range(NCORES)])
    gradd = np.vstack([results[k]["gradd"] for k in range(NCORES)])

    # scatter same-class (diag band) values over the dense outputs
    L = hist[ts]                                   # band length per sorted row
    rows_rep = np.repeat(np.arange(N), L)
    band_off = np.concatenate([[0], np.cumsum(L)])[:-1]
    idx = np.arange(L.sum()) - np.repeat(band_off, L)       # 0..L[r)-1 within row
    jj = cs[ts[rows_rep]] + idx                   # sorted-space column
    kk = jj - W0[rows_rep // 128]                 # column within slab
    loss_p[rows_rep, jj] = lossd[rows_rep, kk]
    grad_p[rows_rep, jj] = gradd[rows_rep, kk]

    if not rv.all():                               # rows with no negatives: loss = 0
        loss_p[rv == 0.0, :] = 0.0

    out_loss = np.empty((N, N), dtype=np.float32)
    out_grad = np.empty((N, N), dtype=np.float32)
    pix = np.ix_(perm, perm)
    out_loss[pix] = loss_p
    out_grad[pix] = grad_p
    return out_loss.reshape(-1), out_grad.reshape(-1)


def run(sim_mat, targets, trace=False):
    from concourse.bass_utils import run_bass_kernel_spmd
    perm, ts, cs, ce, hist, rv, W0, W1, WD, in_maps = _prepare(sim_mat, targets)
    if WD not in _prog_cache:
        _prog_cache[WD] = _build_program(WD)
    nc = _prog_cache[WD]
    res = run_bass_kernel_spmd(nc, in_maps, list(range(NCORES)), trace=trace)
    outs = _assemble(res.results, perm, ts, cs, ce, hist, rv, W0, W1, WD)
    return outs, res.exec_time_ns


def kernel(sim_mat, targets):
    outs, _ = run(sim_mat, targets, trace=False)
    return outs
